# revision 15
# baseline (speedup 1.0000x reference)
"""CompressedLinear kernel for 8 TRN2 NeuronCores.

out[B,S,DOUT] = x[B,S,DIN] @ (w_int8 * scale).T + bias

Strategy (tensor-parallel, per sharding hint):
  - Shard weight rows (DOUT=11008) across 8 cores -> 1376 rows/core.
  - Replicate x to all cores.
  - Host-side prep: fold `scale` into x, cast x and w to fp16 (int8 codes
    <= 127 are exact in fp16), and pre-transpose both operands into
    K-major layouts so every DMA is contiguous per partition line.
  - On-chip: out_tile[128 tok, n] accumulates over K in PSUM via
    matmul(lhsT=xT_tile[128k, 128tok], rhs=wT_tile[128k, n]); epilogue is
    one DVE add (psum + bias_broadcast -> SBUF fp32), then DMA out.
  - Gather: concat per-core outputs along the feature axis on host.

Head/tail scheduling (the PE is >98% busy in steady state, so the only
wins left are at the edges):
  - First-dependency DMAs (k0:2 slices of x-tiles 0/1, n0 slice of w
    chunk 0) are issued from the Scalar/GpSimd/Vector queues, which come
    out of engine boot ~1us before the Sync queue does; the bulk streams
    follow on Sync.  The leading two token tiles consume k0/k1 in
    n-major order so compute can start as soon as those slices land.
  - PE warmup (HAM clock-gate ramp) uses a small memset so it starts as
    early as possible.
  - The last token tile runs n-major with per-n epilogue+store so the
    final store is only the 352-wide slice.
"""

import sys
import types

import numpy as np

import concourse.mybir as mybir
import concourse.tile as tile
from concourse import bacc
from concourse.bass_utils import run_bass_kernel_spmd


def _ensure_ntff_hook():
    """Some images lack antenv.axon_hooks; run_bass_kernel_spmd imports it
    on the traced path (e.g. if BASS_TRACE is set in the environment).
    Register a working shim backed by the axon .so when possible, else a
    no-op getter, so tracing degrades gracefully instead of crashing."""
    try:
        import antenv.axon_hooks  # noqa: F401
        return
    except ImportError:
        pass
    hook = None
    try:
        from trn_agent_boot.trn_boot import _ntff_profile_via_ctypes

        hook = _ntff_profile_via_ctypes("/opt/axon/libaxon_pjrt.so")
    except Exception:
        hook = None
    mod = types.ModuleType("antenv.axon_hooks")
    mod.get_axon_ntff_profile_hook = lambda: hook
    mod.set_axon_ntff_profile_hook = lambda h: None
    sys.modules["antenv.axon_hooks"] = mod


_ensure_ntff_hook()

# Problem shapes (hardcoded per contract)
B, S, DIN, DOUT = 2, 2048, 4096, 11008
NCORES = 8
TOK = B * S                      # 4096 tokens
DSH = DOUT // NCORES             # 1376 output features per core
P = 128
KC = DIN // P                    # 32 contraction chunks of 128
MT = TOK // P                    # 32 token tiles of 128
N_TILE = 512
N_SIZES = (512, 512, 352)        # n-tiles covering DSH=1376
HEAD_KC = 2                      # leading k-slices loaded via early queues
WARM_N = 128                     # warmup matmul width
WARM_COUNT = 28                  # warmup matmuls before the first real one
WARM_FILL = 96                   # gap-filler warmups after the leading k<2 block

_cached = {}


def build_module(mt=MT, kc=KC, dsh=DSH, n_sizes=N_SIZES, num_devices=NCORES):
    """Build + compile the Bass module (same NEFF for all cores)."""
    nc = bacc.Bacc(
        "TRN2",
        target_bir_lowering=False,
        debug=False,
        num_devices=num_devices,
    )
    fp16 = mybir.dt.float16
    fp32 = mybir.dt.float32

    # DRAM I/O (per-core shapes; layouts pre-arranged on host)
    x_d = nc.dram_tensor("x", (mt, P, kc, P), fp16, kind="ExternalInput")
    w_d = nc.dram_tensor("w", (P, kc, dsh), mybir.dt.int8, kind="ExternalInput")
    b_d = nc.dram_tensor("b", (1, dsh), fp32, kind="ExternalInput")
    o_d = nc.dram_tensor("out", (mt, P, dsh), fp32, kind="ExternalOutput")

    n_off = []
    off = 0
    for ns in n_sizes:
        n_off.append(off)
        off += ns
    assert off == dsh

    # Weight DMA chunk boundaries (in kc units).  DMA descriptors span
    # per-partition contiguous bytes and the stream is descriptor-rate
    # limited (~345ns/descriptor/queue-engine), so int8 chunks of 4 kc
    # (5504B descriptors) move 2x the kc per queue-slot that fp16 2-kc
    # chunks did.  The first two chunks stay at 2 kc to cut the head
    # latency (smaller DMA + shorter DVE cast before the first matmul).
    if kc == 32:
        w_bounds = [0, 2, 4, 8, 12, 16, 20, 24, 28, 32]
    else:
        step = 2 if kc % 2 == 0 else 1
        w_bounds = list(range(0, kc + 1, step))
    # kc index -> (chunk index, offset within chunk)
    k2chunk = []
    for ci in range(len(w_bounds) - 1):
        for kk in range(w_bounds[ci + 1] - w_bounds[ci]):
            k2chunk.append((ci, kk))

    # How many leading token-tiles to k-interleave so PE work overlaps the
    # weight load (each tile is ~18.3us of PE work vs ~30us of w DMA).
    n_group = 2 if mt >= 2 else mt
    hkc = HEAD_KC

    with tile.TileContext(nc) as tc:
        with (
            tc.tile_pool(name="wpool", bufs=1) as wpool,
            tc.tile_pool(name="w8pool", bufs=3) as w8pool,
            tc.tile_pool(name="xpool", bufs=4) as xpool,
            tc.tile_pool(name="opool", bufs=3) as opool,
            tc.tile_pool(name="psum", bufs=2, space="PSUM") as psum_pool,
        ):
            # ---- head ------------------------------------------------------
            # PE warmup: dummy matmuls on a small zeroed scratch tile so the
            # HAM clock-gate ramps while the head DMAs are in flight.  The
            # memset is the warmup's gate: GpSimd issues it at ~6.0us as
            # long as nothing else sits ahead of it in that queue.
            # 128-wide so the fill granularity is fine (~60-200ns each).
            warm_src = wpool.tile([P, WARM_N], fp16, tag="warm_src")
            nc.gpsimd.memset(warm_src[:], 0)
            warm_ps = psum_pool.tile([P, WARM_N], fp32, tag="warm", name="warm")
            for _ in range(WARM_COUNT):
                nc.tensor.matmul(
                    warm_ps[:], warm_src[:, :P], warm_src[:], start=True, stop=True
                )

            # x head slices ride the GpSimd queue AFTER the memset: that
            # queue is slow (~18GB/s) but starts at ~6.9us, so 64KB lands
            # ~10.5/12.5us without costing Sync queue issue slots (a Sync
            # dma_start costs ~0.66us of descriptor-gen that delays every
            # later chunk).  w chunk 0 is Sync's first, data ~8.2-11um.
            xheads = []
            for g in range(n_group):
                xh = wpool.tile([P, hkc, P], fp16, tag=f"x{g}h")
                nc.gpsimd.dma_start(out=xh[:], in_=x_d.ap()[g][:, 0:hkc, :])
                xheads.append(xh)

            # Weights ship as int8 (codes <= 127, exact) halving the w
            # stream to 5.65MB, and are upcast to fp16 chunk-by-chunk on
            # the DVE, which is otherwise idle until the first epilogue
            # (~52us).  This removes the mid-head w-chunk deficit (the
            # fp16 stream measured 5-12us of PE stalls at 20-40us).
            def load_w_chunk_i8(lo, hi, tag):
                w8 = w8pool.tile([P, hi - lo, dsh], mybir.dt.int8, tag="w8")
                nc.sync.dma_start(out=w8[:], in_=w_d.ap()[:, lo:hi, :])
                wt = wpool.tile([P, hi - lo, dsh], fp16, tag=tag)
                nc.vector.tensor_copy(out=wt[:], in_=w8[:])
                return wt

            w0full = load_w_chunk_i8(0, hkc, "w0")

            # bias: 5.5KB row on the GpSimd queue (lands ~10us), broadcast
            # on-chip.  As a 704KB pre-broadcast tile at the end of the Sync
            # stream it landed ~52us and gated the first epilogues, which
            # gated psum-ring release for m2 (~2.7us PE stall).
            bias_row = wpool.tile([1, dsh], fp32, tag="bias_row")
            nc.gpsimd.dma_start(out=bias_row[:], in_=b_d.ap())
            bias_sb = wpool.tile([P, dsh], fp32, tag="bias")
            nc.gpsimd.partition_broadcast(bias_sb[:], bias_row[:])

            # ---- bulk streams on Sync --------------------------------------
            def alloc_xm(m):
                xm = xpool.tile([P, kc, P], fp16, tag="xm", name=f"xm{m}")
                nc.sync.dma_start(out=xm[:], in_=x_d.ap()[m])
                return xm

            def alloc_psums(m):
                psums = []
                for n in range(len(n_sizes)):
                    ps_full = psum_pool.tile(
                        [P, N_TILE], fp32, tag=f"ps{n}", name=f"ps{n}_{m}"
                    )
                    psums.append(ps_full[:, : n_sizes[n]])
                return psums

            def w_slice(wt, kk, n):
                return wt[:, kk, n_off[n] : n_off[n] + n_sizes[n]]

            def mm_lhsT(psums, lhsT, k, wt, kk):
                for n in range(len(n_sizes)):
                    nc.tensor.matmul(
                        psums[n],
                        lhsT,
                        w_slice(wt, kk, n),
                        start=(k == 0),
                        stop=(k == kc - 1),
                    )

            def epilogue(m, psums, split_store=False):
                om = opool.tile([P, dsh], fp32, tag="om", name=f"om{m}")
                for n in range(len(n_sizes)):
                    sl = slice(n_off[n], n_off[n] + n_sizes[n])
                    nc.vector.tensor_add(
                        out=om[:, sl], in0=psums[n], in1=bias_sb[:, sl]
                    )
                    if split_store:
                        nc.sync.dma_start(out=o_d.ap()[m][:, sl], in_=om[:, sl])
                if not split_store:
                    nc.sync.dma_start(out=o_d.ap()[m], in_=om[:])

            # Sync issue order: w chunk 1 (2 kc), the two full leading x
            # tiles (k-slicing x only shrinks its descriptors, not their
            # count, so full 8KB-descriptor tiles are strictly better),
            # then the remaining 4-kc chunks in consumption order.
            w_tiles = [w0full]
            w_tiles.append(load_w_chunk_i8(w_bounds[1], w_bounds[2], "w1"))
            group_xms = [alloc_xm(g) for g in range(n_group)]
            for c in range(2, len(w_bounds) - 1):
                w_tiles.append(load_w_chunk_i8(w_bounds[c], w_bounds[c + 1], f"w{c}"))

            def x_lead(g, k):
                if k < hkc:
                    return xheads[g][:, k, :]
                return group_xms[g][:, k, :]


            # Leading group, k < hkc: g-major so g0's matmuls are gated
            # only on (x0h, w0), not on the later-arriving x1h.
            group_psums = [alloc_psums(m) for m in range(n_group)]
            for g in range(n_group):
                for k in range(hkc):
                    for n in range(len(n_sizes)):
                        nc.tensor.matmul(
                            group_psums[g][n],
                            xheads[g][:, k, :],
                            w_slice(w0full, k, n),
                            start=(k == 0),
                            stop=False,
                        )
            # Fill the DMA wait before chunk 1 lands with cheap warmup
            # matmuls so the PE clock-ramp doesn't reset (an idle gap here
            # measured ~2us of half-speed matmuls afterwards).
            for _ in range(WARM_FILL):
                nc.tensor.matmul(
                    warm_ps[:], warm_src[:, :P], warm_src[:], start=True, stop=True
                )

            # Leading group, k >= hkc: interleave over k so matmuls consume
            # weight chunks in arrival order across n_group token tiles.
            # The last few k are de-interleaved (g0 finishes first) so g0's
            # epilogue overlaps g1's tail matmuls and the psum ring buffers
            # are free when m2 starts (interleaved finish measured a 3.4us
            # stall on m2's first matmul waiting for m0's epilogue).
            ksplit = kc - 4
            for k in range(hkc, ksplit):
                ci, kk = k2chunk[k]
                wt = w_tiles[ci]
                for g in range(n_group):
                    mm_lhsT(group_psums[g], x_lead(g, k), k, wt, kk)
            for g in range(n_group):
                for k in range(ksplit, kc):
                    ci, kk = k2chunk[k]
                    mm_lhsT(group_psums[g], x_lead(g, k), k, w_tiles[ci], kk)
                epilogue(g, group_psums[g])

            # Steady state
            for m in range(n_group, mt - 1):
                xm = alloc_xm(m)
                psums = alloc_psums(m)
                for k in range(kc):
                    ci, kk = k2chunk[k]
                    mm_lhsT(psums, xm[:, k, :], k, w_tiles[ci], kk)
                epilogue(m, psums)

            # Last tile: n-major with per-n epilogue+store so the tail after
            # the final matmul is only the 352-wide add + store.
            m = mt - 1
            xm = alloc_xm(m)
            om = opool.tile([P, dsh], fp32, tag="om", name=f"om{m}")
            # (offset, width, psum tag) pieces; n2 is split in two so the
            # final add+store after the last matmul is only 176 wide.
            pieces = [
                (0, 512, "ps0"),
                (512, 512, "ps1"),
                (1024, 176, "ps2"),
                (1200, 176, "ps2"),
            ]
            for pi, (noff, nw, ptag) in enumerate(pieces):
                ps = psum_pool.tile([P, N_TILE], fp32, tag=ptag, name=f"lt{pi}")
                sl = slice(noff, noff + nw)
                for k in range(kc):
                    ci, kk = k2chunk[k]
                    wt = w_tiles[ci]
                    nc.tensor.matmul(
                        ps[:, :nw],
                        xm[:, k, :],
                        wt[:, kk, sl],
                        start=(k == 0),
                        stop=(k == kc - 1),
                    )
                nc.vector.tensor_add(out=om[:, sl], in0=ps[:, :nw], in1=bias_sb[:, sl])
                nc.sync.dma_start(out=o_d.ap()[m][:, sl], in_=om[:, sl])

    nc.compile()
    return nc


def _get_module():
    if "nc" not in _cached:
        # num_devices=1: no collectives anywhere in the kernel, and the
        # per-NEFF sync machinery is cheapest in single-device form; the
        # SPMD launcher still runs the same NEFF on all 8 cores.
        _cached["nc"] = build_module(num_devices=1)
    return _cached["nc"]


def _prep_inputs(x, w_int8, scale, bias):
    """Host-side shard + layout prep. Returns in_maps for the 8 cores."""
    # x: fold scale, cast fp16, reorder to [m, kp, kc, t]
    xs = x.reshape(TOK, DIN).astype(np.float32) * np.float32(scale)
    xp = xs.reshape(MT, P, KC, P)        # [m, t, kc, kp]
    xp = np.ascontiguousarray(xp.transpose(0, 3, 2, 1), dtype=np.float16)

    in_maps = []
    for c in range(NCORES):
        wsh = w_int8[c * DSH : (c + 1) * DSH]          # [dsh, DIN] int32
        wp = wsh.reshape(DSH, KC, P).transpose(2, 1, 0)  # [kp, kc, dsh]
        wp = np.ascontiguousarray(wp).astype(np.int8)  # codes in [-127,127]
        bsh = np.ascontiguousarray(
            bias[c * DSH : (c + 1) * DSH].astype(np.float32).reshape(1, DSH)
        )
        in_maps.append({"x": xp, "w": wp, "b": bsh})
    return in_maps


def kernel(x, w_int8, scale, bias):
    nc = _get_module()
    in_maps = _prep_inputs(
        np.asarray(x), np.asarray(w_int8), np.asarray(scale), np.asarray(bias)
    )
    res = run_bass_kernel_spmd(nc, in_maps, core_ids=list(range(NCORES)))
    outs = [res.results[c]["out"].reshape(TOK, DSH) for c in range(NCORES)]
    full = np.concatenate(outs, axis=1)  # [TOK, DOUT]
    return np.ascontiguousarray(full.reshape(B, S, DOUT), dtype=np.float32)
